# revision 17
# baseline (speedup 1.0000x reference)
"""Masked multi-head self-attention kernel for 8 Trainium2 NeuronCores.

Full module: qkv projection -> causal softmax attention (16 heads) -> out
projection, for x[4, 2048, 1024].

Sharding: core c handles batch b = c//2 and heads h0 = (c%2)*8 .. h0+8.
QKV projection + attention are fully local to a core.  The out projection
contracts over all 16 heads' channels, so the two cores of a batch exchange
their attention outputs with pairwise AllGathers (chunked over heads and
query blocks for overlap) and each computes half of the output columns.
Each core returns out[b][:, half].T (transposed: [512, 2048]); the host
reassembles.  Inputs are re-laid-out per core on the host: x transposed,
qkv weight columns / out-proj rows sliced and permuted to the gather order.
"""

import math
import os
import sys

for _p in ("/opt/trn_rl_repo", "/root/.axon_site/_ro/trn_rl_repo"):
    if os.path.isdir(_p) and _p not in sys.path:
        sys.path.insert(0, _p)
        break

import numpy as np

import concourse.bass as bass
import concourse.mybir as mybir
import concourse.tile as tile
from concourse import bacc
from concourse.bass_utils import run_bass_kernel_spmd

B, T, C, H = 4, 2048, 1024, 16
D = 64                 # head dim
NCORES = 8
HPC = H // 2           # heads per core = 8
CPC = HPC * D          # channels per core = 512
P = 128                # partitions
QB = 512               # query block
NQB = T // QB          # 4
KC = C // P            # contraction chunks for C = 8
NTT = T // P           # 16 t-tiles
SCALE = 1.0 / math.sqrt(D)

F32 = mybir.dt.float32
F32R = mybir.dt.float32r
BF16 = mybir.dt.bfloat16
EXP = mybir.ActivationFunctionType.Exp

_CACHE = {}


def build():
    nc = bacc.Bacc("TRN2", num_devices=NCORES, debug=False)

    xT = nc.dram_tensor("xT", [C, T], F32R, kind="ExternalInput")
    wqkv = nc.dram_tensor("wqkv", [C, 3 * CPC], F32R, kind="ExternalInput")
    bqkv = nc.dram_tensor("bqkv", [1, 3 * CPC], F32, kind="ExternalInput")
    bqv = nc.dram_tensor("bqv", [1, CPC], F32R, kind="ExternalInput")
    wout = nc.dram_tensor("wout", [C, CPC], F32R, kind="ExternalInput")
    bout = nc.dram_tensor("bout", [1, CPC], F32, kind="ExternalInput")
    outT = nc.dram_tensor("outT", [CPC, T], F32, kind="ExternalOutput")

    groups = [[0, 1], [2, 3], [4, 5], [6, 7]]

    with tile.TileContext(nc) as tc:
        with (
            tc.tile_pool(name="const", bufs=1) as constp,
            tc.tile_pool(name="ytp", bufs=1) as ytp,
            tc.tile_pool(name="vaugp", bufs=1) as vaugp,
            tc.tile_pool(name="dram", bufs=1, space="DRAM") as dramp,
        ):
            # per-partition bias layouts: bq_sb[p, n] = bqkv[n*128 + p]
            bq_sb = constp.tile([P, 12], F32, tag="bq")
            nc.sync.dma_start(
                bq_sb[:].rearrange("p (o n) -> p o n", o=1),
                bqkv.ap().rearrange("o (n p) -> p o n", p=P),
            )
            bo_sb = constp.tile([P, 4], F32, tag="bo")
            nc.sync.dma_start(
                bo_sb[:].rearrange("p (o n) -> p o n", o=1),
                bout.ap().rearrange("o (n p) -> p o n", p=P),
            )
            bv_sb = constp.tile([1, CPC], F32R, tag="bv")
            nc.sync.dma_start(bv_sb[:], bqv[0:1, :])
            ones_f32 = constp.tile([P, P], F32, tag="ones")
            nc.vector.memset(ones_f32[:], 1.0)
            onesr = constp.tile([1, P], F32R, tag="onesr")
            nc.vector.tensor_copy(onesr[:], ones_f32[0:1, :])

            # Q^T,K^T: 8 chunks of [128 ch, 2048 t] (Q: 0-3, K: 4-7)
            yts = [
                ytp.tile([P, T], BF16, name=f"yt{n}", tag=f"yt{n}")
                for n in range(8)
            ]
            # V natural (+ones col) per head: ktile k at cols k*65
            vaugs = [
                vaugp.tile([P, NTT * 65], BF16, name=f"vaug{h}", tag=f"vaug{h}")
                for h in range(HPC)
            ]
            vaug3s = [
                v[:].rearrange("p (k c) -> p k c", c=65) for v in vaugs
            ]
            for h in range(HPC):
                nc.vector.tensor_copy(
                    vaug3s[h][:, :, 64:65],
                    ones_f32[:, 0:NTT].rearrange("p (a b) -> p a b", b=1),
                )

            # ---------------- stage 1: qkv projection, V ----------------
            with (
                tc.tile_pool(name="xtp", bufs=1) as xtp,
                tc.tile_pool(name="wtile", bufs=10) as wtp,
                tc.tile_pool(name="wvp", bufs=1) as wvp,
                tc.tile_pool(name="ps_y", bufs=6, space="PSUM") as psy,
                tc.tile_pool(name="ps_v", bufs=2, space="PSUM") as psv,
            ):
                # x^T chunks resident in SBUF: [128 ch, 2048 t] each
                xt = xtp.tile([P, KC * T], F32R, tag="xt")
                xt3 = xt[:].rearrange("p (c t) -> p c t", t=T)
                for cc in range(KC):
                    nc.sync.dma_start(
                        xt3[:, cc, :], xT[cc * P:(cc + 1) * P, :]
                    )

                # Q^T, K^T: kc outer so each weight tile serves 4 matmuls
                for n in range(8):
                    pys = [
                        psy.tile([P, QB], F32, name=f"py{n}_{i}", tag="py")
                        for i in range(4)
                    ]
                    for kc in range(KC):
                        wt = wtp.tile([P, P], F32R, tag="wt")
                        nc.sync.dma_start(
                            wt[:],
                            wqkv[kc * P:(kc + 1) * P, n * P:(n + 1) * P],
                        )
                        for tc4 in range(4):
                            nc.tensor.matmul(
                                pys[tc4][:],
                                wt[:],
                                xt3[:, kc, tc4 * QB:(tc4 + 1) * QB],
                                start=(kc == 0),
                                stop=(kc == KC - 1),
                            )
                    for tc4 in range(4):
                        nc.vector.tensor_scalar_add(
                            yts[n][:, tc4 * QB:(tc4 + 1) * QB],
                            pys[tc4][:],
                            bq_sb[:, n:n + 1],
                        )

                # V natural: out[t, vch] with x^T tiles stationary
                wv_tiles = []
                for kc in range(KC):
                    wv = wvp.tile(
                        [P, CPC], F32R, name=f"wv{kc}", tag=f"wv{kc}"
                    )
                    nc.sync.dma_start(
                        wv[:], wqkv[kc * P:(kc + 1) * P, 2 * CPC:3 * CPC]
                    )
                    wv_tiles.append(wv)
                for tt in range(NTT):
                    pv = psv.tile([P, CPC], F32, tag="pv")
                    nc.tensor.matmul(
                        pv[:], onesr[0:1, :], bv_sb[0:1, :],
                        start=True, stop=False,
                    )
                    for kc in range(KC):
                        nc.tensor.matmul(
                            pv[:],
                            xt3[:, kc, tt * P:(tt + 1) * P],
                            wv_tiles[kc][:],
                            start=False,
                            stop=(kc == KC - 1),
                        )
                    # scatter per-head columns into vaug tiles
                    for h in range(HPC):
                        nc.vector.tensor_copy(
                            vaug3s[h][:, tt, 0:64],
                            pv[:, h * 64:h * 64 + 64],
                        )

            # ---------------- stage 2+3: attention, gather, out proj ----
            with (
                tc.tile_pool(name="pt", bufs=36) as ptp,
                tc.tile_pool(name="recip", bufs=4) as recipp,
                tc.tile_pool(name="bc", bufs=3) as bcp,
                tc.tile_pool(name="atv", bufs=3) as atvp,
                tc.tile_pool(name="w2", bufs=1) as w2p,
                tc.tile_pool(name="agr", bufs=3) as agrp,
                tc.tile_pool(name="outsb", bufs=3) as outsbp,
                tc.tile_pool(name="ps_s", bufs=4, space="PSUM") as pss,
                tc.tile_pool(name="ps_a", bufs=2, space="PSUM") as psa,
                tc.tile_pool(name="ps_o", bufs=2, space="PSUM") as pso,
            ):
                w2sb = w2p.tile([P, KC * CPC], F32R, tag="w2")
                nc.sync.dma_start(
                    w2sb[:].rearrange("p (c n) -> p c n", n=CPC),
                    wout.ap().rearrange("(c p) n -> p c n", p=P),
                )
                w23 = w2sb[:].rearrange("p (c n) -> p c n", n=CPC)

                def s_pass(qb, h):
                    """score matmuls + exp (+causal mask) for one head/qblock.
                    Diagonal k-tiles first so their exp+mask (on the PV
                    critical path) complete while off-diagonal scores stream.
                    """
                    qt = yts[h // 2]
                    kt_c = yts[4 + h // 2]
                    poff = (h % 2) * 64
                    nkt = 4 * qb + 4
                    kts = list(range(4 * qb, nkt)) + list(range(0, 4 * qb))
                    pts = []
                    for kt in kts:
                        j = kt - 4 * qb  # >=0 on diagonal tiles
                        qoff = max(j, 0) * P
                        ps = pss.tile([P, QB], F32, tag="ps")
                        nc.tensor.matmul(
                            ps[:, qoff:QB],
                            kt_c[poff:poff + 64, kt * P:(kt + 1) * P],
                            qt[poff:poff + 64, qb * QB + qoff:(qb + 1) * QB],
                            start=True, stop=True,
                        )
                        pt = ptp.tile([P, QB], BF16, tag="pt")
                        nc.scalar.activation(
                            pt[:, qoff:QB], ps[:, qoff:QB], EXP, scale=SCALE
                        )
                        if j >= 0:
                            # zero where q < k (also fills the stale prefix)
                            nc.gpsimd.affine_select(
                                out=pt[:],
                                in_=pt[:],
                                compare_op=mybir.AluOpType.is_ge,
                                fill=0.0,
                                base=-j * P,
                                pattern=[[1, QB]],
                                channel_multiplier=-1,
                            )
                        pts.append((kt, pt))
                    return pts

                def pv_pass(qb, h, pts, ag_in, row):
                    pa = psa.tile([P, QB], F32, tag="pa")
                    for i, (kt, pt) in enumerate(pts):
                        nc.tensor.matmul(
                            pa[0:65, :],
                            vaug3s[h][:, kt, :],
                            pt[:],
                            start=(i == 0),
                            stop=(i == len(pts) - 1),
                        )
                    sums = recipp.tile([1, QB], F32, tag="sums")
                    nc.vector.tensor_copy(sums[:], pa[64:65, :])
                    recip = recipp.tile([1, QB], F32, tag="recip")
                    nc.vector.reciprocal_approx_fast(recip[:], sums[:])
                    bc = bcp.tile([64, QB], F32, tag="bc")
                    nc.gpsimd.partition_broadcast(bc[:], recip[:])
                    atv = atvp.tile([64, QB], F32R, tag="atv")
                    nc.vector.tensor_mul(atv[:], pa[0:64, :], bc[:])
                    nc.sync.dma_start(
                        ag_in[row * 64:(row + 1) * 64, :], atv[:]
                    )

                def gather(ag_in, ag_out):
                    nc.gpsimd.collective_compute(
                        "AllGather",
                        mybir.AluOpType.bypass,
                        replica_groups=groups,
                        ins=[ag_in.opt()],
                        outs=[ag_out.opt()],
                    )

                def out_proj(qb, ag_outs):
                    # w_out rows are host-permuted to match the gathered
                    # row order [even0-3, odd0-3, even4-5, odd4-5, ...]
                    agr3s = []
                    for gi, ago in enumerate(ag_outs):
                        ncch = 2 * (GGRP[gi][1] - GGRP[gi][0]) * 64 // P
                        agr = agrp.tile(
                            [P, ncch * QB], F32R,
                            name=f"agr{qb}_{gi}", tag=f"agr{gi}",
                        )
                        nc.sync.dma_start(
                            agr[:].rearrange("p (c n) -> p c n", n=QB),
                            ago[:].rearrange("(c p) n -> p c n", p=P),
                        )
                        agr3s.append(
                            agr[:].rearrange("p (c n) -> p c n", n=QB)
                        )
                    # chunk cc -> (gather buffer, sub-chunk)
                    ccmap = [(0, 0), (0, 1), (0, 2), (0, 3),
                             (1, 0), (1, 1), (2, 0), (2, 1)]
                    for oc in range(4):
                        po = pso.tile([P, QB], F32, tag="po")
                        for cc in range(KC):
                            gi, sub = ccmap[cc]
                            nc.tensor.matmul(
                                po[:],
                                w23[:, cc, oc * P:(oc + 1) * P],
                                agr3s[gi][:, sub, :],
                                start=(cc == 0),
                                stop=(cc == KC - 1),
                            )
                        osb = outsbp.tile([P, QB], F32, tag="osb")
                        nc.vector.tensor_scalar_add(
                            osb[:], po[:], bo_sb[:, oc:oc + 1]
                        )
                        nc.sync.dma_start(
                            outT[oc * P:(oc + 1) * P, qb * QB:(qb + 1) * QB],
                            osb[:],
                        )

                # gather groups: heads 0-3, heads 4-5, heads 6-7
                GGRP = [(0, 4), (4, 6), (6, 8)]

                pending_outproj = None
                for qb in range(NQB):
                    ag_ins = [
                        dramp.tile(
                            [(e - s) * 64, QB], F32R,
                            name=f"agin{qb}_{i}", tag=f"agin{qb}_{i}",
                        )
                        for i, (s, e) in enumerate(GGRP)
                    ]
                    ag_outs = [
                        dramp.tile(
                            [2 * (e - s) * 64, QB], F32R,
                            name=f"agout{qb}_{i}", tag=f"agout{qb}_{i}",
                        )
                        for i, (s, e) in enumerate(GGRP)
                    ]
                    grp_of = {}
                    for i, (s, e) in enumerate(GGRP):
                        for h in range(s, e):
                            grp_of[h] = (i, h - s)
                    prev = None
                    for h in range(HPC):
                        cur = s_pass(qb, h)
                        if h == 3 and pending_outproj is not None:
                            # previous qblock's out-projection: its gather
                            # waits hide behind this qblock's score matmuls
                            pending_outproj()
                            pending_outproj = None
                        if prev is not None:
                            hp = h - 1
                            gi, row = grp_of[hp]
                            pv_pass(qb, hp, prev, ag_ins[gi], row)
                            if hp in (3, 5):
                                gather(ag_ins[gi], ag_outs[gi])
                        prev = cur
                    gi, row = grp_of[HPC - 1]
                    pv_pass(qb, HPC - 1, prev, ag_ins[gi], row)
                    gather(ag_ins[gi], ag_outs[gi])
                    pending_outproj = (
                        lambda qb=qb, ag_outs=ag_outs: out_proj(qb, ag_outs)
                    )
                pending_outproj()

    nc.compile()
    return nc


def kernel(x, w_qkv, b_qkv, w_out, b_out):
    x = np.asarray(x, dtype=np.float32)
    w_qkv = np.asarray(w_qkv, dtype=np.float32)
    b_qkv = np.asarray(b_qkv, dtype=np.float32)
    w_out = np.asarray(w_out, dtype=np.float32)
    b_out = np.asarray(b_out, dtype=np.float32)

    if "nc" not in _CACHE:
        _CACHE["nc"] = build()
    nc = _CACHE["nc"]

    in_maps = []
    for c in range(NCORES):
        b = c // 2
        h0 = (c % 2) * HPC
        cols = slice(h0 * D, h0 * D + CPC)
        wq = np.concatenate(
            [w_qkv[:, cols], w_qkv[:, C:][:, cols], w_qkv[:, 2 * C:][:, cols]],
            axis=1,
        )
        bq = np.concatenate(
            [b_qkv[cols], b_qkv[C:][cols], b_qkv[2 * C:][cols]]
        ).reshape(1, 3 * CPC)
        half = slice((c % 2) * CPC, (c % 2) * CPC + CPC)
        wo = w_out[:, half]
        # rows permuted to the gathered channel order:
        # [even h0-3, odd h0-3, even h4-5, odd h4-5, even h6-7, odd h6-7]
        wo_perm = np.concatenate(
            [wo[0:256], wo[512:768],
             wo[256:384], wo[768:896],
             wo[384:512], wo[896:1024]], axis=0
        )
        in_maps.append({
            "xT": np.ascontiguousarray(x[b].T),
            "wqkv": np.ascontiguousarray(wq),
            "bqkv": np.ascontiguousarray(bq),
            "bqv": np.ascontiguousarray(bq[:, 2 * CPC:3 * CPC]),
            "wout": np.ascontiguousarray(wo_perm),
            "bout": np.ascontiguousarray(b_out[half]).reshape(1, CPC),
        })

    kwargs = {}
    tdir = os.environ.get("KERNEL_TRACE_DIR")
    if tdir:
        kwargs = dict(trace=True, tmpdir=tdir)
    res = run_bass_kernel_spmd(
        nc, in_maps, core_ids=list(range(NCORES)), **kwargs
    )
    _CACHE["last_results"] = res

    out = np.empty((B, T, C), dtype=np.float32)
    for c in range(NCORES):
        b = c // 2
        half = slice((c % 2) * CPC, (c % 2) * CPC + CPC)
        out[b][:, half] = res.results[c]["outT"].T
    return out


# revision 20
# speedup vs baseline: 1.0855x; 1.0855x over previous
"""Masked multi-head self-attention kernel for 8 Trainium2 NeuronCores.

Full module: qkv projection -> causal softmax attention (16 heads) -> out
projection, for x[4, 2048, 1024].

Sharding: core c handles batch b = c//2 and heads h0 = (c%2)*8 .. h0+8.
QKV projection + attention are fully local to a core.  The out projection
contracts over all 16 heads' channels, so the two cores of a batch exchange
their attention outputs with pairwise AllGathers (chunked over heads and
query blocks for overlap) and each computes half of the output columns.
Each core returns out[b][:, half].T (transposed: [512, 2048]); the host
reassembles.  Inputs are re-laid-out per core on the host: x transposed,
qkv weight columns / out-proj rows sliced and permuted to the gather order.
"""

import math
import os
import sys

for _p in ("/opt/trn_rl_repo", "/root/.axon_site/_ro/trn_rl_repo"):
    if os.path.isdir(_p) and _p not in sys.path:
        sys.path.insert(0, _p)
        break

import ml_dtypes
import numpy as np

import concourse.bass as bass
import concourse.mybir as mybir
import concourse.tile as tile
from concourse import bacc
from concourse.bass_utils import run_bass_kernel_spmd

B, T, C, H = 4, 2048, 1024, 16
D = 64                 # head dim
NCORES = 8
HPC = H // 2           # heads per core = 8
CPC = HPC * D          # channels per core = 512
P = 128                # partitions
QB = 512               # query block
NQB = T // QB          # 4
KC = C // P            # contraction chunks for C = 8
NTT = T // P           # 16 t-tiles
SCALE = 1.0 / math.sqrt(D)

F32 = mybir.dt.float32
F32R = mybir.dt.float32r
BF16 = mybir.dt.bfloat16
EXP = mybir.ActivationFunctionType.Exp

_CACHE = {}


def build():
    nc = bacc.Bacc("TRN2", num_devices=NCORES, debug=False)

    xT = nc.dram_tensor("xT", [C, T], F32R, kind="ExternalInput")
    wqkv = nc.dram_tensor("wqkv", [C, 3 * CPC], F32R, kind="ExternalInput")
    bqkv = nc.dram_tensor("bqkv", [1, 3 * CPC], F32, kind="ExternalInput")
    wout = nc.dram_tensor("wout", [C, CPC], BF16, kind="ExternalInput")
    bout = nc.dram_tensor("bout", [1, CPC], F32, kind="ExternalInput")
    outT = nc.dram_tensor("outT", [CPC, T], F32, kind="ExternalOutput")

    groups = [[0, 1], [2, 3], [4, 5], [6, 7]]

    with tile.TileContext(nc) as tc:
        with (
            tc.tile_pool(name="const", bufs=1) as constp,
            tc.tile_pool(name="ytp", bufs=1) as ytp,
            tc.tile_pool(name="vaugp", bufs=1) as vaugp,
            tc.tile_pool(name="dram", bufs=1, space="DRAM") as dramp,
        ):
            # per-partition bias layouts: bq_sb[p, n] = bqkv[n*128 + p]
            bq_sb = constp.tile([P, 12], F32, tag="bq")
            nc.sync.dma_start(
                bq_sb[:].rearrange("p (o n) -> p o n", o=1),
                bqkv.ap().rearrange("o (n p) -> p o n", p=P),
            )
            bo_sb = constp.tile([P, 4], F32, tag="bo")
            nc.sync.dma_start(
                bo_sb[:].rearrange("p (o n) -> p o n", o=1),
                bout.ap().rearrange("o (n p) -> p o n", p=P),
            )
            ones_f32 = constp.tile([P, P], F32, tag="ones")
            nc.vector.memset(ones_f32[:], 1.0)

            # Q^T,K^T: 8 chunks of [128 ch, 2048 t] (Q: 0-3, K: 4-7)
            yts = [
                ytp.tile([P, T], BF16, name=f"yt{n}", tag=f"yt{n}")
                for n in range(8)
            ]
            # V natural (+ones col) per head: ktile k at cols k*65
            vaugs = [
                vaugp.tile([P, NTT * 65], BF16, name=f"vaug{h}", tag=f"vaug{h}")
                for h in range(HPC)
            ]
            vaug3s = [
                v[:].rearrange("p (k c) -> p k c", c=65) for v in vaugs
            ]
            for h in range(HPC):
                nc.vector.tensor_copy(
                    vaug3s[h][:, :, 64:65],
                    ones_f32[:, 0:NTT].rearrange("p (a b) -> p a b", b=1),
                )

            # ---------------- stage 1: qkv projection, V ----------------
            with (
                tc.tile_pool(name="xtp", bufs=1) as xtp,
                tc.tile_pool(name="wtile", bufs=10) as wtp,
                tc.tile_pool(name="wvp", bufs=1) as wvp,
                tc.tile_pool(name="ps_y", bufs=4, space="PSUM") as psy,
                tc.tile_pool(name="ps_v", bufs=4, space="PSUM") as psv,
            ):
                # x^T chunks resident in SBUF: [128 ch, 2048 t] each
                xts = [
                    xtp.tile([P, T], F32R, name=f"xt{cc}", tag=f"xt{cc}")
                    for cc in range(KC)
                ]
                for cc in range(KC):
                    nc.sync.dma_start(
                        xts[cc][:], xT[cc * P:(cc + 1) * P, :]
                    )

                wv_tiles = []
                for kc in range(KC):
                    wv = wvp.tile(
                        [P, CPC], F32R, name=f"wv{kc}", tag=f"wv{kc}"
                    )
                    nc.sync.dma_start(
                        wv[:], wqkv[kc * P:(kc + 1) * P, 2 * CPC:3 * CPC]
                    )
                    wv_tiles.append(wv)

                def qk_chunk(n):
                    # kc outer so each weight tile serves 4 matmuls
                    pys = [
                        psy.tile([P, QB], F32, name=f"py{n}_{i}", tag="py")
                        for i in range(4)
                    ]
                    for kc in range(KC):
                        wt = wtp.tile([P, P], F32R, tag="wt")
                        nc.sync.dma_start(
                            wt[:],
                            wqkv[kc * P:(kc + 1) * P, n * P:(n + 1) * P],
                        )
                        for tc4 in range(4):
                            nc.tensor.matmul(
                                pys[tc4][:],
                                wt[:],
                                xts[kc][:, tc4 * QB:(tc4 + 1) * QB],
                                start=(kc == 0),
                                stop=(kc == KC - 1),
                            )
                    for tc4 in range(4):
                        nc.vector.tensor_scalar_add(
                            yts[n][:, tc4 * QB:(tc4 + 1) * QB],
                            pys[tc4][:],
                            bq_sb[:, n:n + 1],
                        )

                def v_block(tts):
                    # V natural: out[t, vch] with x^T tiles stationary;
                    # V bias is folded into the output bias on the host
                    for tt in tts:
                        pv = psv.tile([P, CPC], F32, tag="pv")
                        for kc in range(KC):
                            nc.tensor.matmul(
                                pv[:],
                                xts[kc][:, tt * P:(tt + 1) * P],
                                wv_tiles[kc][:],
                                start=(kc == 0),
                                stop=(kc == KC - 1),
                            )
                        for h in range(HPC):
                            nc.vector.tensor_copy(
                                vaug3s[h][:, tt, 0:64],
                                pv[:, h * 64:h * 64 + 64],
                            )

                for blk in range(4):
                    qk_chunk(blk)
                    qk_chunk(4 + blk)
                    v_block(range(4 * blk, 4 * blk + 4))

            # ---------------- stage 2+3: attention, gather, out proj ----
            with (
                tc.tile_pool(name="pt", bufs=36) as ptp,
                tc.tile_pool(name="recip", bufs=4) as recipp,
                tc.tile_pool(name="bc", bufs=3) as bcp,
                tc.tile_pool(name="atv", bufs=3) as atvp,
                tc.tile_pool(name="w2", bufs=1) as w2p,
                tc.tile_pool(name="agr", bufs=3) as agrp,
                tc.tile_pool(name="outsb", bufs=3) as outsbp,
                tc.tile_pool(name="ps_s", bufs=4, space="PSUM") as pss,
                tc.tile_pool(name="ps_a", bufs=2, space="PSUM") as psa,
                tc.tile_pool(name="ps_o", bufs=2, space="PSUM") as pso,
            ):
                w2sb = w2p.tile([P, KC * CPC], BF16, tag="w2")
                nc.sync.dma_start(
                    w2sb[:].rearrange("p (c n) -> p c n", n=CPC),
                    wout.ap().rearrange("(c p) n -> p c n", p=P),
                )
                w23 = w2sb[:].rearrange("p (c n) -> p c n", n=CPC)

                def s_pass(qb, h):
                    """score matmuls + exp (+causal mask) for one head/qblock.
                    Diagonal k-tiles first so their exp+mask (on the PV
                    critical path) complete while off-diagonal scores stream.
                    """
                    qt = yts[h // 2]
                    kt_c = yts[4 + h // 2]
                    poff = (h % 2) * 64
                    nkt = 4 * qb + 4
                    kts = list(range(4 * qb, nkt)) + list(range(0, 4 * qb))
                    pts = []
                    for kt in kts:
                        j = kt - 4 * qb  # >=0 on diagonal tiles
                        qoff = max(j, 0) * P
                        ps = pss.tile([P, QB], F32, tag="ps")
                        nc.tensor.matmul(
                            ps[:, qoff:QB],
                            kt_c[poff:poff + 64, kt * P:(kt + 1) * P],
                            qt[poff:poff + 64, qb * QB + qoff:(qb + 1) * QB],
                            start=True, stop=True,
                        )
                        pt = ptp.tile([P, QB], BF16, tag="pt")
                        nc.scalar.activation(
                            pt[:, qoff:QB], ps[:, qoff:QB], EXP, scale=SCALE
                        )
                        if j >= 0:
                            # zero where q < k (also fills the stale prefix)
                            nc.gpsimd.affine_select(
                                out=pt[:],
                                in_=pt[:],
                                compare_op=mybir.AluOpType.is_ge,
                                fill=0.0,
                                base=-j * P,
                                pattern=[[1, QB]],
                                channel_multiplier=-1,
                            )
                        pts.append((kt, pt))
                    return pts

                def pv_pass(qb, h, pts, ag_in, row):
                    pa = psa.tile([P, QB], F32, tag="pa")
                    for i, (kt, pt) in enumerate(pts):
                        nc.tensor.matmul(
                            pa[0:65, :],
                            vaug3s[h][:, kt, :],
                            pt[:],
                            start=(i == 0),
                            stop=(i == len(pts) - 1),
                        )
                    sums = recipp.tile([1, QB], F32, tag="sums")
                    nc.vector.tensor_copy(sums[:], pa[64:65, :])
                    recip = recipp.tile([1, QB], F32, tag="recip")
                    nc.vector.reciprocal_approx_fast(recip[:], sums[:])
                    bc = bcp.tile([64, QB], F32, tag="bc")
                    nc.gpsimd.partition_broadcast(bc[:], recip[:])
                    atv = atvp.tile([64, QB], BF16, tag="atv")
                    nc.vector.tensor_mul(atv[:], pa[0:64, :], bc[:])
                    nc.sync.dma_start(
                        ag_in[row * 64:(row + 1) * 64, :], atv[:]
                    )

                def gather(ag_in, ag_out):
                    nc.gpsimd.collective_compute(
                        "AllGather",
                        mybir.AluOpType.bypass,
                        replica_groups=groups,
                        ins=[ag_in.opt()],
                        outs=[ag_out.opt()],
                    )

                def out_proj(qb, ag_outs):
                    # w_out rows are host-permuted to match the gathered
                    # row order [even0-3, odd0-3, even4-5, odd4-5, ...]
                    agr3s = []
                    for gi, ago in enumerate(ag_outs):
                        ncch = 2 * (GGRP[gi][1] - GGRP[gi][0]) * 64 // P
                        agr = agrp.tile(
                            [P, ncch * QB], BF16,
                            name=f"agr{qb}_{gi}", tag=f"agr{gi}",
                        )
                        nc.sync.dma_start(
                            agr[:].rearrange("p (c n) -> p c n", n=QB),
                            ago[:].rearrange("(c p) n -> p c n", p=P),
                        )
                        agr3s.append(
                            agr[:].rearrange("p (c n) -> p c n", n=QB)
                        )
                    # chunk cc -> (gather buffer, sub-chunk)
                    ccmap = [(0, 0), (0, 1), (0, 2), (0, 3),
                             (1, 0), (1, 1), (2, 0), (2, 1)]
                    for oc in range(4):
                        po = pso.tile([P, QB], F32, tag="po")
                        for cc in range(KC):
                            gi, sub = ccmap[cc]
                            nc.tensor.matmul(
                                po[:],
                                w23[:, cc, oc * P:(oc + 1) * P],
                                agr3s[gi][:, sub, :],
                                start=(cc == 0),
                                stop=(cc == KC - 1),
                            )
                        osb = outsbp.tile([P, QB], F32, tag="osb")
                        nc.vector.tensor_scalar_add(
                            osb[:], po[:], bo_sb[:, oc:oc + 1]
                        )
                        nc.sync.dma_start(
                            outT[oc * P:(oc + 1) * P, qb * QB:(qb + 1) * QB],
                            osb[:],
                        )

                # gather groups: heads 0-3, heads 4-5, heads 6-7
                GGRP = [(0, 4), (4, 6), (6, 8)]

                pending_outproj = None
                for qb in range(NQB):
                    ag_ins = [
                        dramp.tile(
                            [(e - s) * 64, QB], BF16,
                            name=f"agin{qb}_{i}", tag=f"agin{qb}_{i}",
                        )
                        for i, (s, e) in enumerate(GGRP)
                    ]
                    ag_outs = [
                        dramp.tile(
                            [2 * (e - s) * 64, QB], BF16,
                            name=f"agout{qb}_{i}", tag=f"agout{qb}_{i}",
                        )
                        for i, (s, e) in enumerate(GGRP)
                    ]
                    grp_of = {}
                    for i, (s, e) in enumerate(GGRP):
                        for h in range(s, e):
                            grp_of[h] = (i, h - s)
                    prev = None
                    for h in range(HPC):
                        cur = s_pass(qb, h)
                        if h == 3 and pending_outproj is not None:
                            # previous qblock's out-projection: its gather
                            # waits hide behind this qblock's score matmuls
                            pending_outproj()
                            pending_outproj = None
                        if prev is not None:
                            hp = h - 1
                            gi, row = grp_of[hp]
                            pv_pass(qb, hp, prev, ag_ins[gi], row)
                            if hp in (3, 5):
                                gather(ag_ins[gi], ag_outs[gi])
                        prev = cur
                    gi, row = grp_of[HPC - 1]
                    pv_pass(qb, HPC - 1, prev, ag_ins[gi], row)
                    gather(ag_ins[gi], ag_outs[gi])
                    pending_outproj = (
                        lambda qb=qb, ag_outs=ag_outs: out_proj(qb, ag_outs)
                    )
                pending_outproj()

    nc.compile()
    return nc


def kernel(x, w_qkv, b_qkv, w_out, b_out):
    x = np.asarray(x, dtype=np.float32)
    w_qkv = np.asarray(w_qkv, dtype=np.float32)
    b_qkv = np.asarray(b_qkv, dtype=np.float32)
    w_out = np.asarray(w_out, dtype=np.float32)
    b_out = np.asarray(b_out, dtype=np.float32)

    if "nc" not in _CACHE:
        _CACHE["nc"] = build()
    nc = _CACHE["nc"]

    # V bias passes through softmax unchanged; fold it into the out bias
    bv_perm_all = b_qkv[2 * C:3 * C]

    in_maps = []
    for c in range(NCORES):
        b = c // 2
        h0 = (c % 2) * HPC
        cols = slice(h0 * D, h0 * D + CPC)
        wq = np.concatenate(
            [w_qkv[:, cols], w_qkv[:, C:][:, cols], w_qkv[:, 2 * C:][:, cols]],
            axis=1,
        )
        bq = np.concatenate(
            [b_qkv[cols], b_qkv[C:][cols], b_qkv[2 * C:][cols]]
        ).reshape(1, 3 * CPC)
        half = slice((c % 2) * CPC, (c % 2) * CPC + CPC)
        wo = w_out[:, half]
        # rows permuted to the gathered channel order:
        # [even h0-3, odd h0-3, even h4-5, odd h4-5, even h6-7, odd h6-7]
        wo_perm = np.concatenate(
            [wo[0:256], wo[512:768],
             wo[256:384], wo[768:896],
             wo[384:512], wo[896:1024]], axis=0
        )
        bout_eff = b_out[half] + bv_perm_all @ w_out[:, half]
        in_maps.append({
            "xT": np.ascontiguousarray(x[b].T),
            "wqkv": np.ascontiguousarray(wq),
            "bqkv": np.ascontiguousarray(bq),
            "wout": np.ascontiguousarray(wo_perm.astype(ml_dtypes.bfloat16)),
            "bout": np.ascontiguousarray(bout_eff).reshape(1, CPC),
        })

    kwargs = {}
    tdir = os.environ.get("KERNEL_TRACE_DIR")
    if tdir:
        kwargs = dict(trace=True, tmpdir=tdir)
    res = run_bass_kernel_spmd(
        nc, in_maps, core_ids=list(range(NCORES)), **kwargs
    )
    _CACHE["last_results"] = res

    out = np.empty((B, T, C), dtype=np.float32)
    for c in range(NCORES):
        b = c // 2
        half = slice((c % 2) * CPC, (c % 2) * CPC + CPC)
        out[b][:, half] = res.results[c]["outT"].T
    return out
